# revision 54
# baseline (speedup 1.0000x reference)
"""Multi-head self-attention on 8 trn2 NeuronCores.

Problem: B=4, S=2048, E=1024, H=8, D=128 MHA with a boolean attention mask.

Sharding: batch x head-group. Core c computes batch b=c//2 for heads
[4*(c%2), 4*(c%2)+4). Each core produces a partial output [S, E] (its 4
heads' contribution through w_out); the host sums the two partials per
batch. No on-device collectives needed.

Device algorithm (per core), everything in "transposed" layout so that the
attention*V contraction needs no on-chip transpose of the softmax matrix:
  phase 1 (all heads): QT/KT/VT[h] = w[h].T @ qT  (PE, [D=128, S] tiles),
    V[h] = transpose(VT[h]) via PE transpose-mode, [S-keys, D].
  phase 2 is one flat stream over (head, pair, kt) units with the logits
  matmuls emitted one unit ahead (so ACT never waits on the PE at a pair
  boundary). Per unit:
    lgT[128k, 1024q] = KT-tile.T @ QT  (2 matmuls sharing the KT weights)
    expT = exp(scale * lgT)            (one ScalarE op, bf16 out)
    expT *= keepT-tile                 (one VectorE op; masked keys -> 0)
    denominator: 12 of the 16 key tiles accumulate elementwise on the DVE
    (bf16 adds into accD); the 4 tiles in PE_SET go straight to the PE as
    [128,2]-selector matmuls into one [2,512] PSUM tile (row = query half),
    where accD is also folded in at kt 15. This costs the PE 10 tiny
    matmuls/pair instead of the 32 [1,512] ones-matmuls a per-kt PE
    denominator would need, and keeps DVE/ACT/PE within ~3% of each other.
    (GpSimd is left idle on purpose: Pool tensor ops share SBUF ports with
    the DVE and slow concurrent DVE ops ~4x.)
    av += V-tile.T @ expT              (PE, [128D, 512q] x2, accumulated)
    pair tail: one LN + one EXP(-x) on the [2,512] sums (both live in the
    natural_log_exp ACT table set), then deferred one pair: [2,128]-selector
    matmuls broadcast the reciprocal rows onto 128 partitions and the DVE
    normalizes straight out of PSUM: headsT = avs * pb.
  phase 3: out[128q, E] = sum_h headsT[h].T @ w_out[h]  (fp16 to DRAM,
    PSUM evacuation split ACT/DVE, 4-deep fp16 staging so the 512KB output
    DMAs never stall the PE)

exp is computed without a running row-max: logits here are ~N(0, 2.7^2), so
exp stays well inside fp32 range and softmax is shift invariant.

Perf history (HW exec): 379.0us baseline -> ~304.5us: removed the per-kt
[1,512] denominator matmuls (-62us PE), flattened phase 2 with lg lookahead
(-15us of ACT starvation), 4-deep fp16 output staging (-10us), startup DMA
fan-out across SP+ACT sequencers and early first-tile slices (-4us).
Known dead ends: fp8 anywhere (error budget is 2e-2, fp8 costs ~5% rms),
GpSimd offload (shares SBUF ports with DVE, 4x slowdown), custom-DVE fast
reciprocal (walrus codegen rejects it), lg lookahead 2 (needs 6 PSUM banks).
"""

import math

import ml_dtypes
import numpy as np

import concourse.bass as bass
import concourse.tile as tile
from concourse import mybir
from concourse.bass_utils import run_bass_kernel_spmd
from concourse.masks import make_identity
from concourse.vector_clock import ScopedClock, VectorClock

B, S, E, H, D = 4, 2048, 1024, 8, 128
HPC = 4          # heads per core
NCORES = 8
NKT = S // 128   # key tiles per sequence
NET = E // 128   # contraction tiles for the projections
NQT = S // 128   # query tiles for the output projection
SCALE = 1.0 / math.sqrt(D)
BF16 = mybir.dt.bfloat16
F32 = mybir.dt.float32
F16 = mybir.dt.float16
EXP = mybir.ActivationFunctionType.Exp
LN = mybir.ActivationFunctionType.Ln

_patched = False


def _patch_drain():
    """The installed walrus rejects >1 sem wait on the Tile tail Drain.
    Emit one drain per pending logical processor instead."""
    global _patched
    if _patched:
        return
    _patched = True

    def _drain_and_barrier(self, tick_clock, wait_clock):
        nc = self.nc
        ticks = list(tick_clock.global_clock)
        procs = [i for i, t in enumerate(ticks) if t > 0]
        for p in procs or [None]:
            vec = [0] * len(ticks)
            if p is not None:
                vec[p] = ticks[p]
            d = nc.sync.drain()
            wait_clock.add_sem_waits(d.ins, ScopedClock({None: VectorClock(vec)}))
        nc.all_engine_barrier()
        popped = nc._tile_sem_poison_stack.pop()
        assert popped is self._sem_poison
        nc.clear_and_free_semaphores(list(self.sems.allocated().values()))
        nc.all_engine_barrier()

    tile.TileContext._drain_and_barrier = _drain_and_barrier


def _split_waits(nc):
    """This walrus build only encodes ONE sem wait per instruction. Move
    extra waits onto preceding same-engine NoOps (engines execute their
    instructions in block order, so this is semantically identical)."""
    import bass_rust

    k = 0
    for f in nc.m.functions:
        for bb in f.blocks:
            out = []
            for inst in bb.instructions:
                si = inst.sync_info
                if si is not None and si.on_wait and len(si.on_wait) > 1:
                    waits = list(si.on_wait)
                    for w in waits[:-1]:
                        nop = bass_rust.InstNoOp(
                            name=f"I-waitsplit-{k}", ins=[], outs=[]
                        )
                        k += 1
                        nop.engine = inst.engine
                        nop.sync_info = mybir.SyncInfo(on_wait=[w], on_update=[])
                        out.append(nop)
                    inst.sync_info = mybir.SyncInfo(
                        on_wait=[waits[-1]], on_update=si.on_update
                    )
                out.append(inst)
            bb.instructions[:] = out


_nc_cache = None


def _build_nc():
    global _nc_cache
    if _nc_cache is not None:
        return _nc_cache
    _patch_drain()

    nc = bass.Bass()
    qT_d = nc.declare_dram_parameter("qT", [E, S], BF16, isOutput=False)
    keepT_d = nc.declare_dram_parameter("keepT", [S, S], BF16, isOutput=False)
    wq_d = nc.declare_dram_parameter("wq", [HPC, E, D], BF16, isOutput=False)
    wk_d = nc.declare_dram_parameter("wk", [HPC, E, D], BF16, isOutput=False)
    wv_d = nc.declare_dram_parameter("wv", [HPC, E, D], BF16, isOutput=False)
    wo_d = nc.declare_dram_parameter("wo", [HPC, D, E], BF16, isOutput=False)
    out_d = nc.declare_dram_parameter("out", [S, E], F16, isOutput=True)

    keepT_ap = keepT_d[:, :].rearrange("(kt p) q -> p kt q", p=128)

    with tile.TileContext(nc) as tc:
        with (
            tc.tile_pool(name="const", bufs=1) as constp,
            tc.tile_pool(name="wo", bufs=1) as wop,
            tc.tile_pool(name="hT", bufs=1) as hTp,
            tc.tile_pool(name="qkv", bufs=1) as qkvp,
            tc.tile_pool(name="keeplo", bufs=1) as keeplop,
            tc.tile_pool(name="expt", bufs=12) as expp,
            tc.tile_pool(name="accs", bufs=1) as accp,
            tc.tile_pool(name="small", bufs=2) as smallp,
            tc.tile_pool(name="avs", bufs=2) as avsp,
            tc.tile_pool(name="outs", bufs=4) as outsp,
            tc.tile_pool(name="ps_a", bufs=2, space="PSUM") as ps_a,
            tc.tile_pool(name="ps_av", bufs=2, space="PSUM") as ps_av,
            tc.tile_pool(name="ps_sum", bufs=2, space="PSUM") as ps_sum,
        ):
            # ---- constants ----
            ident = constp.tile([128, 128], BF16)
            make_identity(nc, ident)
            # sums selectors: [128,2] lhsT writing the two 512-query halves
            # into rows 0/1 of one [2,512] PSUM tile
            sumsL_a = constp.tile([128, 2], BF16)
            nc.vector.memset(sumsL_a, 0.0)
            nc.vector.memset(sumsL_a[:, 0:1], 1.0)
            sumsL_b = constp.tile([128, 2], BF16)
            nc.vector.memset(sumsL_b, 0.0)
            nc.vector.memset(sumsL_b[:, 1:2], 1.0)
            # broadcast selectors: [2,128] lhsT replicating row 0/1 of rcb
            # onto all 128 partitions
            bcL_a = constp.tile([2, 128], BF16)
            nc.vector.memset(bcL_a, 0.0)
            nc.vector.memset(bcL_a[0:1, :], 1.0)
            bcL_b = constp.tile([2, 128], BF16)
            nc.vector.memset(bcL_b, 1.0)
            nc.vector.memset(bcL_b[0:1, :], 0.0)

            # w_out: [p(D), h, e] - loaded late (needed only in phase 3)
            wo_s = wop.tile([128, HPC, E], BF16)

            headsT_s = hTp.tile([128, HPC, S], BF16)
            # per-head QT/KT (as [D, S]) and V (as [S-keys, D] in 16 tiles)
            QT_a = [qkvp.tile([128, S], BF16, tag=f"QT{h}", name=f"QT{h}") for h in range(HPC)]
            KT_a = [qkvp.tile([128, S], BF16, tag=f"KT{h}", name=f"KT{h}") for h in range(HPC)]
            V_a = [qkvp.tile([128, NKT, 128], BF16, tag=f"V{h}", name=f"V{h}") for h in range(HPC)]
            # first half of keepT lives alongside qT; second half reuses the
            # SBUF the phase-1 pools release
            keep_lo = keeplop.tile([128, NKT // 2, S], BF16)

            # ================= phase 1: projections, all heads =============
            with (
                tc.tile_pool(name="wqkv", bufs=1) as wqkvp,
                tc.tile_pool(name="qTp", bufs=1) as qTp,
                tc.tile_pool(name="vt", bufs=2) as vtstp,
            ):
                # weights as [p(E-within-tile), h*NET+kt, d]; DMAs ordered by
                # first use: wq[h0], the first qT half, wk/wv[h0], the rest
                w_s = {}
                w_aps = {}
                for name, wd in (("wq", wq_d), ("wk", wk_d), ("wv", wv_d)):
                    w_s[name] = wqkvp.tile(
                        [128, HPC * NET, D], BF16, tag=name, name=name
                    )
                    w_aps[name] = wd[:, :, :].rearrange(
                        "h (kt p) d -> p (h kt) d", p=128
                    )

                def load_w(name, h):
                    nc.sync.dma_start(
                        out=w_s[name][:, h * NET : (h + 1) * NET, :],
                        in_=w_aps[name][:, h * NET : (h + 1) * NET, :],
                    )

                qT_s = qTp.tile([128, NET, S], BF16)
                qT_ap = qT_d[:, :].rearrange("(kt p) s -> p kt s", p=128)

                def load_qT(st2):
                    sl = slice(st2 * 1024, (st2 + 1) * 1024)
                    for kt in range(NET):
                        nc.sync.dma_start(out=qT_s[:, kt, sl], in_=qT_ap[:, kt, sl])

                # warm up the PE p-state during the initial DMA wait: ~3us of
                # dummy identity matmuls ramp the clock 0.65 -> 2.4GHz so the
                # first real projections run at full speed
                warm = ps_sum.tile([128, 512], F32, tag="ps_sum", name="warm")

                def emit_warm(n):
                    for i in range(n):
                        nc.tensor.matmul(
                            warm[:, 0:128], lhsT=ident, rhs=ident,
                            start=(i == 0), stop=(i == n - 1),
                        )

                emit_warm(40)

                # first matmul needs only wq[h0,et0] + qT[et0, first half]:
                # stage those slices first, and fan the startup DMA issue
                # across four idle engine sequencers (the SP sequencer alone
                # takes ~565ns per dma_start, serializing the warm-up)
                nc.sync.dma_start(
                    out=w_s["wq"][:, 0:1, :], in_=w_aps["wq"][:, 0:1, :]
                )
                nc.scalar.dma_start(out=qT_s[:, 0, 0:1024], in_=qT_ap[:, 0, 0:1024])
                nc.scalar.dma_start(out=qT_s[:, 1, 0:1024], in_=qT_ap[:, 1, 0:1024])
                nc.sync.dma_start(
                    out=w_s["wq"][:, 1:NET, :], in_=w_aps["wq"][:, 1:NET, :]
                )
                for kt in range(2, NET):
                    eng = (nc.sync, nc.scalar)[kt % 2]
                    eng.dma_start(out=qT_s[:, kt, 0:1024], in_=qT_ap[:, kt, 0:1024])
                nc.sync.dma_start(
                    out=w_s["wk"][:, 0:NET, :], in_=w_aps["wk"][:, 0:NET, :]
                )
                nc.scalar.dma_start(
                    out=w_s["wv"][:, 0:NET, :], in_=w_aps["wv"][:, 0:NET, :]
                )
                load_qT(1)
                for h in range(1, HPC):
                    for name in ("wq", "wk", "wv"):
                        load_w(name, h)

                # V transposes are deferred one projection unit so the PE
                # never waits on the DVE cast that feeds them
                pending_vt = None

                def _emit_transposes(vt, h, st2):
                    # phase 1 borrows the (otherwise idle) ps_sum slots so the
                    # transpose batches don't contend with projection tiles
                    pst = ps_sum.tile([128, 8, 128], BF16, tag="ps_sum")
                    for j in range(8):
                        nc.tensor.transpose(
                            pst[:, j, :], vt[:, j * 128 : (j + 1) * 128], ident
                        )
                    nc.vector.tensor_copy(V_a[h][:, st2 * 8 : (st2 + 1) * 8, :], pst)

                def _proj(ws_name, h, q0, out_ps):
                    ws = w_s[ws_name]
                    for kt in range(NET):
                        for half in range(2):
                            nc.tensor.matmul(
                                out_ps[:, half * 512 : (half + 1) * 512],
                                lhsT=ws[:, h * NET + kt, :],
                                rhs=qT_s[
                                    :, kt, q0 + half * 512 : q0 + (half + 1) * 512
                                ],
                                start=(kt == 0),
                                stop=(kt == NET - 1),
                            )

                for h in range(HPC):
                    # head 0 runs all st2=0 units first: the st2=1 qT DMAs
                    # are still in flight when the PE starts
                    if h == 0:
                        order = [(w, s) for s in range(2) for w in ("wq", "wk", "wv")]
                    else:
                        order = [(w, s) for w in ("wq", "wk", "wv") for s in range(2)]
                    for wi, (wname, st2) in enumerate(order):
                        q0 = st2 * 1024
                        ps = ps_a.tile([128, 1024], F32, tag="ps_a")
                        _proj(wname, h, q0, ps)
                        if h == 0 and wi < 3:
                            # keep the PE busy between the DMA-paced first
                            # units so the p-state never drops back down
                            emit_warm(20)
                        if wname == "wv":
                            # V: VT pair-tiles then PE-transpose in batches
                            if pending_vt is not None:
                                _emit_transposes(*pending_vt)
                            vt = vtstp.tile([128, 1024], BF16, tag="vt")
                            nc.scalar.copy(vt, ps)
                            pending_vt = (vt, h, st2)
                            continue
                        if pending_vt is not None:
                            _emit_transposes(*pending_vt)
                            pending_vt = None
                        dst = QT_a[h] if wname == "wq" else KT_a[h]
                        # alternate evacuations across ACT and DVE
                        if wi % 2 == 0:
                            nc.scalar.copy(dst[:, q0 : q0 + 1024], ps)
                        else:
                            nc.vector.tensor_copy(dst[:, q0 : q0 + 1024], ps)
                    if h == 0:
                        # stream the first half of keepT during phase 1
                        for kt in range(NKT // 2):
                            nc.sync.dma_start(
                                out=keep_lo[:, kt, :], in_=keepT_ap[:, kt, :]
                            )
                    if h == 1:
                        # w_out is needed only in phase 3
                        nc.sync.dma_start(
                            out=wo_s, in_=wo_d[:, :, :].rearrange("h d e -> d h e")
                        )
                if pending_vt is not None:
                    _emit_transposes(*pending_vt)
                    pending_vt = None

            # ============== phase 2: attention, all heads ==================
            with tc.tile_pool(name="keephi", bufs=1) as keephip:
                keep_hi = keephip.tile([128, NKT // 2, S], BF16)
                for kt in range(NKT // 2):
                    nc.sync.dma_start(
                        out=keep_hi[:, kt, :], in_=keepT_ap[:, NKT // 2 + kt, :]
                    )

                def keep_slice(kt, q0, w):
                    t = keep_lo if kt < NKT // 2 else keep_hi
                    return t[:, kt % (NKT // 2), q0 : q0 + w]

                # deferred normalization chain (one query-group pair deep):
                # pending = (avs0, avs1, rcb, h, q0); rcb is the [2,512]
                # bf16 reciprocal-of-sums (row r = query half r)
                pending = None

                def _finish_pe(item):
                    # broadcast rcb rows onto 128 partitions via selector mms
                    avs0, avs1, rcb, h, q0 = item
                    pb0 = ps_sum.tile([128, 512], F32, tag="ps_sum", name="pb0")
                    nc.tensor.matmul(pb0, lhsT=bcL_a, rhs=rcb, start=True, stop=True)
                    pb1 = ps_sum.tile([128, 512], F32, tag="ps_sum", name="pb1")
                    nc.tensor.matmul(pb1, lhsT=bcL_b, rhs=rcb, start=True, stop=True)
                    return pb0, pb1

                def _finish_dve(item, pb0, pb1):
                    # multiply straight out of the broadcast PSUM tiles; this
                    # runs at 32-bit DVE rate but saves the two rb copies
                    avs0, avs1, rcb, h, q0 = item
                    nc.vector.tensor_mul(headsT_s[:, h, q0 : q0 + 512], avs0, pb0)
                    nc.vector.tensor_mul(
                        headsT_s[:, h, q0 + 512 : q0 + 1024], avs1, pb1
                    )

                # flat stream over (head, pair, kt) units; logits are emitted
                # one unit ahead so the PE is never the reason ACT idles at a
                # pair boundary
                PE_SET = {5, 8, 11, 14}  # key tiles summed on the PE

                def unit(u):
                    return u // 32, (u // 16) % 2, u % 16

                lg_of = {}

                def emit_lg(u):
                    h, pair, kt = unit(u)
                    q0 = pair * 1024
                    lg = ps_a.tile([128, 1024], F32, tag="ps_a", name="lg")
                    for half in range(2):
                        nc.tensor.matmul(
                            lg[:, half * 512 : (half + 1) * 512],
                            lhsT=KT_a[h][:, kt * 128 : (kt + 1) * 128],
                            rhs=QT_a[h][:, q0 + half * 512 : q0 + (half + 1) * 512],
                            start=True,
                            stop=True,
                        )
                    lg_of[u] = lg

                NU = HPC * 2 * NKT
                av0 = av1 = sm2 = accD = exD_first = None
                haveD = False
                emit_lg(0)
                for u in range(NU):
                    h, pair, kt = unit(u)
                    q0 = pair * 1024
                    if kt == 0:
                        av0 = ps_av.tile([128, 512], F32, tag="ps_av", name="av0")
                        av1 = ps_av.tile([128, 512], F32, tag="ps_av", name="av1")
                        accD = accp.tile(
                            [128, 1024], BF16, tag="accD", name="accD", bufs=2
                        )
                        exD_first = None
                        haveD = False
                    if u + 1 < NU:
                        emit_lg(u + 1)
                    if kt == 1 and pending is not None:
                        pbs = _finish_pe(pending)
                    lg = lg_of.pop(u)
                    ex = expp.tile([128, 1024], BF16, tag="ex", name="ex")
                    nc.scalar.activation(ex, lg, EXP, scale=SCALE)
                    nc.vector.tensor_mul(ex, ex, keep_slice(kt, q0, 1024))
                    if kt == 1 and pending is not None:
                        _finish_dve(pending, *pbs)
                        pending = None
                    # denominator: DVE accumulates the non-PE_SET key tiles
                    if kt not in PE_SET:
                        if exD_first is None:
                            exD_first = ex
                        elif not haveD:
                            nc.vector.tensor_add(accD, exD_first, ex)
                            haveD = True
                        else:
                            nc.vector.tensor_add(accD, accD, ex)
                    first, last = kt == 0, kt == NKT - 1
                    nc.tensor.matmul(
                        av0, lhsT=V_a[h][:, kt, :], rhs=ex[:, 0:512],
                        start=first, stop=last,
                    )
                    nc.tensor.matmul(
                        av1, lhsT=V_a[h][:, kt, :], rhs=ex[:, 512:1024],
                        start=first, stop=last,
                    )
                    if kt in PE_SET:
                        if kt == min(PE_SET):
                            # allocated late: the ps_sum ring slot must first
                            # rotate through the previous pair's pb tiles
                            sm2 = ps_sum.tile([2, 512], F32, tag="ps_sum", name="sm2")
                        nc.tensor.matmul(
                            sm2, lhsT=sumsL_a, rhs=ex[:, 0:512],
                            start=(kt == min(PE_SET)), stop=False,
                        )
                        nc.tensor.matmul(
                            sm2, lhsT=sumsL_b, rhs=ex[:, 512:1024],
                            start=False, stop=False,
                        )
                    if kt == NKT - 1:
                        # fold the DVE accumulator into the PSUM sums
                        nc.tensor.matmul(
                            sm2, lhsT=sumsL_a, rhs=accD[:, 0:512],
                            start=False, stop=False,
                        )
                        nc.tensor.matmul(
                            sm2, lhsT=sumsL_b, rhs=accD[:, 512:1024],
                            start=False, stop=True,
                        )
                        # reciprocal via ln + exp(-x), one [2,512] ACT op each
                        lnsm = smallp.tile(
                            [2, 512], F32, tag="lnsm", name="lnsm", bufs=1
                        )
                        nc.scalar.activation(lnsm, sm2, LN)
                        rcb = smallp.tile([2, 512], BF16, tag="rcb", name="rcb")
                        nc.scalar.activation(rcb, lnsm, EXP, scale=-1.0)
                        # evacuate accumulators promptly (frees PSUM banks)
                        avs0 = avsp.tile([128, 512], BF16, tag="avs", name="avs0")
                        avs1 = avsp.tile([128, 512], BF16, tag="avs", name="avs1")
                        nc.scalar.copy(avs0, av0)
                        nc.vector.tensor_copy(avs1, av1)
                        pending = (avs0, avs1, rcb, h, q0)
                if pending is not None:
                    pbs = _finish_pe(pending)
                    _finish_dve(pending, *pbs)
                    pending = None

                # ============== phase 3: output projection =================
                for qt in range(NQT):
                    po = ps_a.tile([128, 1024], F32, tag="ps_a")
                    poA, poB = po[:, 0:512], po[:, 512:1024]
                    for h in range(HPC):
                        lh = headsT_s[:, h, qt * 128 : (qt + 1) * 128]
                        for half, dst in ((0, poA), (1, poB)):
                            nc.tensor.matmul(
                                dst,
                                lhsT=lh,
                                rhs=wo_s[:, h, half * 512 : (half + 1) * 512],
                                start=(h == 0),
                                stop=(h == HPC - 1),
                            )
                    ob = outsp.tile([128, E], F16, tag="ob")
                    # split the evacuation across ACT and DVE so the PSUM
                    # banks release twice as fast
                    nc.scalar.copy(ob[:, 0:512], poA)
                    nc.vector.tensor_copy(ob[:, 512:1024], poB)
                    # two half DMAs: the first half ships while the second
                    # half is still evacuating
                    nc.sync.dma_start(
                        out=out_d[qt * 128 : (qt + 1) * 128, 0:512],
                        in_=ob[:, 0:512],
                    )
                    nc.sync.dma_start(
                        out=out_d[qt * 128 : (qt + 1) * 128, 512:1024],
                        in_=ob[:, 512:1024],
                    )

    _split_waits(nc)
    _nc_cache = nc
    return nc


def kernel(q, mask, w_query, w_key, w_value, w_out):
    nc = _build_nc()
    bf16 = ml_dtypes.bfloat16

    qT = np.ascontiguousarray(np.transpose(q.astype(bf16), (0, 2, 1)))
    keepT = np.ascontiguousarray(np.transpose((~mask).astype(bf16), (0, 2, 1)))
    wq = np.ascontiguousarray(w_query.astype(bf16))
    wk = np.ascontiguousarray(w_key.astype(bf16))
    wv = np.ascontiguousarray(w_value.astype(bf16))
    wo = np.ascontiguousarray(w_out.astype(bf16))

    in_maps = []
    for c in range(NCORES):
        b, g = c // 2, c % 2
        hs = slice(g * HPC, (g + 1) * HPC)
        in_maps.append(
            {
                "qT": qT[b],
                "keepT": keepT[b],
                "wq": wq[hs],
                "wk": wk[hs],
                "wv": wv[hs],
                "wo": wo[hs],
            }
        )

    global _last_in_maps
    _last_in_maps = in_maps
    res = run_bass_kernel_spmd(nc, in_maps, list(range(NCORES)))
    outs = [r["out"].astype(np.float32) for r in res.results]
    return np.stack([outs[2 * b] + outs[2 * b + 1] for b in range(B)])


# revision 57
# speedup vs baseline: 1.0256x; 1.0256x over previous
"""Multi-head self-attention on 8 trn2 NeuronCores.

Problem: B=4, S=2048, E=1024, H=8, D=128 MHA with a boolean attention mask.

Sharding: batch x head-group. Core c computes batch b=c//2 for heads
[4*(c%2), 4*(c%2)+4). Each core produces a partial output [S, E] (its 4
heads' contribution through w_out); the host sums the two partials per
batch. No on-device collectives needed.

Device algorithm (per core), everything in "transposed" layout so that the
attention*V contraction needs no on-chip transpose of the softmax matrix:
  phase 1 (all heads): QT/KT/VT[h] = w[h].T @ qT  (PE, [D=128, S] tiles),
    V[h] = transpose(VT[h]) via PE transpose-mode, [S-keys, D].
  phase 2 is one flat stream over (head, pair, kt) units with the logits
  matmuls emitted one unit ahead (so ACT never waits on the PE at a pair
  boundary). Per unit:
    lgT[128k, 1024q] = KT-tile.T @ QT  (2 matmuls sharing the KT weights)
    expT = exp(scale * lgT)            (one ScalarE op, bf16 out)
    expT *= keepT-tile                 (one VectorE op; masked keys -> 0)
    denominator: 12 of the 16 key tiles accumulate elementwise on the DVE
    (bf16 adds into accD); the 4 tiles in PE_SET go straight to the PE as
    [128,2]-selector matmuls into one [2,512] PSUM tile (row = query half),
    where accD is also folded in at kt 15. This costs the PE 10 tiny
    matmuls/pair instead of the 32 [1,512] ones-matmuls a per-kt PE
    denominator would need, and keeps DVE/ACT/PE within ~3% of each other.
    (GpSimd is left idle on purpose: Pool tensor ops share SBUF ports with
    the DVE and slow concurrent DVE ops ~4x.)
    av += V-tile.T @ expT              (PE, [128D, 512q] x2, accumulated)
    pair tail: one LN + one EXP(-x) on the [2,512] sums (both live in the
    natural_log_exp ACT table set), then deferred one pair: [2,128]-selector
    matmuls broadcast the reciprocal rows onto 128 partitions and the DVE
    normalizes straight out of PSUM: headsT = avs * pb.
  phase 3: out[128q, E] = sum_h headsT[h].T @ w_out[h]  (fp16 to DRAM,
    PSUM evacuation split ACT/DVE, 4-deep fp16 staging so the 512KB output
    DMAs never stall the PE)

exp is computed without a running row-max: logits here are ~N(0, 2.7^2), so
exp stays well inside fp32 range and softmax is shift invariant.

Perf history (HW exec): 379.0us baseline -> ~304.5us: removed the per-kt
[1,512] denominator matmuls (-62us PE), flattened phase 2 with lg lookahead
(-15us of ACT starvation), 4-deep fp16 output staging (-10us), startup DMA
fan-out across SP+ACT sequencers and early first-tile slices (-4us).
Known dead ends: fp8 anywhere (error budget is 2e-2, fp8 costs ~5% rms),
GpSimd offload (shares SBUF ports with DVE, 4x slowdown), custom-DVE fast
reciprocal (walrus codegen rejects it), lg lookahead 2 (needs 6 PSUM banks).
"""

import math

import ml_dtypes
import numpy as np

import concourse.bass as bass
import concourse.tile as tile
from concourse import mybir
from concourse.bass_utils import run_bass_kernel_spmd
from concourse.masks import make_identity
from concourse.vector_clock import ScopedClock, VectorClock

B, S, E, H, D = 4, 2048, 1024, 8, 128
HPC = 4          # heads per core
NCORES = 8
NKT = S // 128   # key tiles per sequence
NET = E // 128   # contraction tiles for the projections
NQT = S // 128   # query tiles for the output projection
SCALE = 1.0 / math.sqrt(D)
BF16 = mybir.dt.bfloat16
F32 = mybir.dt.float32
F16 = mybir.dt.float16
EXP = mybir.ActivationFunctionType.Exp
LN = mybir.ActivationFunctionType.Ln

_patched = False


def _patch_drain():
    """The installed walrus rejects >1 sem wait on the Tile tail Drain.
    Emit one drain per pending logical processor instead."""
    global _patched
    if _patched:
        return
    _patched = True

    def _drain_and_barrier(self, tick_clock, wait_clock):
        nc = self.nc
        ticks = list(tick_clock.global_clock)
        procs = [i for i, t in enumerate(ticks) if t > 0]
        for p in procs or [None]:
            vec = [0] * len(ticks)
            if p is not None:
                vec[p] = ticks[p]
            d = nc.sync.drain()
            wait_clock.add_sem_waits(d.ins, ScopedClock({None: VectorClock(vec)}))
        nc.all_engine_barrier()
        popped = nc._tile_sem_poison_stack.pop()
        assert popped is self._sem_poison
        nc.clear_and_free_semaphores(list(self.sems.allocated().values()))
        nc.all_engine_barrier()

    tile.TileContext._drain_and_barrier = _drain_and_barrier


def _split_waits(nc):
    """This walrus build only encodes ONE sem wait per instruction. Move
    extra waits onto preceding same-engine NoOps (engines execute their
    instructions in block order, so this is semantically identical)."""
    import bass_rust

    k = 0
    for f in nc.m.functions:
        for bb in f.blocks:
            out = []
            for inst in bb.instructions:
                si = inst.sync_info
                if si is not None and si.on_wait and len(si.on_wait) > 1:
                    waits = list(si.on_wait)
                    for w in waits[:-1]:
                        nop = bass_rust.InstNoOp(
                            name=f"I-waitsplit-{k}", ins=[], outs=[]
                        )
                        k += 1
                        nop.engine = inst.engine
                        nop.sync_info = mybir.SyncInfo(on_wait=[w], on_update=[])
                        out.append(nop)
                    inst.sync_info = mybir.SyncInfo(
                        on_wait=[waits[-1]], on_update=si.on_update
                    )
                out.append(inst)
            bb.instructions[:] = out


_nc_cache = None


def _build_nc():
    global _nc_cache
    if _nc_cache is not None:
        return _nc_cache
    _patch_drain()

    nc = bass.Bass()
    qT_d = nc.declare_dram_parameter("qT", [E, S], BF16, isOutput=False)
    keepT_d = nc.declare_dram_parameter("keepT", [S, S], BF16, isOutput=False)
    wq_d = nc.declare_dram_parameter("wq", [HPC, E, D], BF16, isOutput=False)
    wk_d = nc.declare_dram_parameter("wk", [HPC, E, D], BF16, isOutput=False)
    wv_d = nc.declare_dram_parameter("wv", [HPC, E, D], BF16, isOutput=False)
    wo_d = nc.declare_dram_parameter("wo", [HPC, D, E], BF16, isOutput=False)
    out_d = nc.declare_dram_parameter("out", [S, E], F16, isOutput=True)

    keepT_ap = keepT_d[:, :].rearrange("(kt p) q -> p kt q", p=128)

    with tile.TileContext(nc) as tc:
        with (
            tc.tile_pool(name="const", bufs=1) as constp,
            tc.tile_pool(name="wo", bufs=1) as wop,
            tc.tile_pool(name="hT", bufs=1) as hTp,
            tc.tile_pool(name="qkv", bufs=1) as qkvp,
            tc.tile_pool(name="keeplo", bufs=1) as keeplop,
            tc.tile_pool(name="expt", bufs=12) as expp,
            tc.tile_pool(name="accs", bufs=1) as accp,
            tc.tile_pool(name="small", bufs=2) as smallp,
            tc.tile_pool(name="avs", bufs=2) as avsp,
            tc.tile_pool(name="outs", bufs=4) as outsp,
            tc.tile_pool(name="ps_a", bufs=2, space="PSUM") as ps_a,
            tc.tile_pool(name="ps_av", bufs=2, space="PSUM") as ps_av,
            tc.tile_pool(name="ps_sum", bufs=2, space="PSUM") as ps_sum,
        ):
            # ---- constants ----
            ident = constp.tile([128, 128], BF16)
            make_identity(nc, ident)
            # sums selectors: [128,2] lhsT writing the two 512-query halves
            # into rows 0/1 of one [2,512] PSUM tile
            sumsL_a = constp.tile([128, 2], BF16)
            nc.vector.memset(sumsL_a, 0.0)
            nc.vector.memset(sumsL_a[:, 0:1], 1.0)
            sumsL_b = constp.tile([128, 2], BF16)
            nc.vector.memset(sumsL_b, 0.0)
            nc.vector.memset(sumsL_b[:, 1:2], 1.0)
            # broadcast selectors: [2,128] lhsT replicating row 0/1 of rcb
            # onto all 128 partitions
            bcL_a = constp.tile([2, 128], BF16)
            nc.vector.memset(bcL_a, 0.0)
            nc.vector.memset(bcL_a[0:1, :], 1.0)
            bcL_b = constp.tile([2, 128], BF16)
            nc.vector.memset(bcL_b, 1.0)
            nc.vector.memset(bcL_b[0:1, :], 0.0)

            # w_out: [p(D), h, e] - loaded late (needed only in phase 3)
            wo_s = wop.tile([128, HPC, E], BF16)

            headsT_s = hTp.tile([128, HPC, S], BF16)
            # per-head QT/KT (as [D, S]) and V (as [S-keys, D] in 16 tiles)
            QT_a = [qkvp.tile([128, S], BF16, tag=f"QT{h}", name=f"QT{h}") for h in range(HPC)]
            KT_a = [qkvp.tile([128, S], BF16, tag=f"KT{h}", name=f"KT{h}") for h in range(HPC)]
            V_a = [qkvp.tile([128, NKT, 128], BF16, tag=f"V{h}", name=f"V{h}") for h in range(HPC)]
            # first half of keepT lives alongside qT; second half reuses the
            # SBUF the phase-1 pools release
            keep_lo = keeplop.tile([128, NKT // 2, S], BF16)

            # ================= phase 1: projections, all heads =============
            with (
                tc.tile_pool(name="wqkv", bufs=1) as wqkvp,
                tc.tile_pool(name="qTp", bufs=1) as qTp,
                tc.tile_pool(name="vt", bufs=2) as vtstp,
            ):
                # weights as [p(E-within-tile), h*NET+kt, d]; DMAs ordered by
                # first use: wq[h0], the first qT half, wk/wv[h0], the rest
                w_s = {}
                w_aps = {}
                for name, wd in (("wq", wq_d), ("wk", wk_d), ("wv", wv_d)):
                    w_s[name] = wqkvp.tile(
                        [128, HPC * NET, D], BF16, tag=name, name=name
                    )
                    w_aps[name] = wd[:, :, :].rearrange(
                        "h (kt p) d -> p (h kt) d", p=128
                    )

                def load_w(name, h):
                    nc.sync.dma_start(
                        out=w_s[name][:, h * NET : (h + 1) * NET, :],
                        in_=w_aps[name][:, h * NET : (h + 1) * NET, :],
                    )

                qT_s = qTp.tile([128, NET, S], BF16)
                qT_ap = qT_d[:, :].rearrange("(kt p) s -> p kt s", p=128)

                def load_qT(st2):
                    sl = slice(st2 * 1024, (st2 + 1) * 1024)
                    for kt in range(NET):
                        nc.sync.dma_start(out=qT_s[:, kt, sl], in_=qT_ap[:, kt, sl])

                # first matmul needs only wq[h0,et0] + qT[et0, first half]:
                # stage those slices first, and fan the startup DMA issue
                # across four idle engine sequencers (the SP sequencer alone
                # takes ~565ns per dma_start, serializing the warm-up)
                nc.sync.dma_start(
                    out=w_s["wq"][:, 0:1, :], in_=w_aps["wq"][:, 0:1, :]
                )
                nc.scalar.dma_start(out=qT_s[:, 0, 0:1024], in_=qT_ap[:, 0, 0:1024])
                nc.scalar.dma_start(out=qT_s[:, 1, 0:1024], in_=qT_ap[:, 1, 0:1024])
                nc.sync.dma_start(
                    out=w_s["wq"][:, 1:NET, :], in_=w_aps["wq"][:, 1:NET, :]
                )
                for kt in range(2, NET):
                    eng = (nc.sync, nc.scalar)[kt % 2]
                    eng.dma_start(out=qT_s[:, kt, 0:1024], in_=qT_ap[:, kt, 0:1024])
                nc.sync.dma_start(
                    out=w_s["wk"][:, 0:NET, :], in_=w_aps["wk"][:, 0:NET, :]
                )
                nc.scalar.dma_start(
                    out=w_s["wv"][:, 0:NET, :], in_=w_aps["wv"][:, 0:NET, :]
                )
                load_qT(1)
                for h in range(1, HPC):
                    for name in ("wq", "wk", "wv"):
                        load_w(name, h)

                # V transposes are deferred one projection unit so the PE
                # never waits on the DVE cast that feeds them
                pending_vt = None

                def _emit_transposes(vt, h, st2):
                    # phase 1 borrows the (otherwise idle) ps_sum slots so the
                    # transpose batches don't contend with projection tiles
                    pst = ps_sum.tile([128, 8, 128], BF16, tag="ps_sum")
                    for j in range(8):
                        nc.tensor.transpose(
                            pst[:, j, :], vt[:, j * 128 : (j + 1) * 128], ident
                        )
                    nc.vector.tensor_copy(V_a[h][:, st2 * 8 : (st2 + 1) * 8, :], pst)

                def _proj(ws_name, h, q0, out_ps):
                    ws = w_s[ws_name]
                    for kt in range(NET):
                        for half in range(2):
                            nc.tensor.matmul(
                                out_ps[:, half * 512 : (half + 1) * 512],
                                lhsT=ws[:, h * NET + kt, :],
                                rhs=qT_s[
                                    :, kt, q0 + half * 512 : q0 + (half + 1) * 512
                                ],
                                start=(kt == 0),
                                stop=(kt == NET - 1),
                            )

                for h in range(HPC):
                    # head 0 runs all st2=0 units first: the st2=1 qT DMAs
                    # are still in flight when the PE starts
                    if h == 0:
                        order = [(w, s) for s in range(2) for w in ("wq", "wk", "wv")]
                    else:
                        order = [(w, s) for w in ("wq", "wk", "wv") for s in range(2)]
                    for wi, (wname, st2) in enumerate(order):
                        q0 = st2 * 1024
                        ps = ps_a.tile([128, 1024], F32, tag="ps_a")
                        _proj(wname, h, q0, ps)
                        if wname == "wv":
                            # V: VT pair-tiles then PE-transpose in batches
                            if pending_vt is not None:
                                _emit_transposes(*pending_vt)
                            vt = vtstp.tile([128, 1024], BF16, tag="vt")
                            nc.scalar.copy(vt, ps)
                            pending_vt = (vt, h, st2)
                            continue
                        if pending_vt is not None:
                            _emit_transposes(*pending_vt)
                            pending_vt = None
                        dst = QT_a[h] if wname == "wq" else KT_a[h]
                        # alternate evacuations across ACT and DVE
                        if wi % 2 == 0:
                            nc.scalar.copy(dst[:, q0 : q0 + 1024], ps)
                        else:
                            nc.vector.tensor_copy(dst[:, q0 : q0 + 1024], ps)
                    if h == 0:
                        # stream the first half of keepT during phase 1
                        for kt in range(NKT // 2):
                            nc.sync.dma_start(
                                out=keep_lo[:, kt, :], in_=keepT_ap[:, kt, :]
                            )
                    if h == 1:
                        # w_out is needed only in phase 3
                        nc.sync.dma_start(
                            out=wo_s, in_=wo_d[:, :, :].rearrange("h d e -> d h e")
                        )
                if pending_vt is not None:
                    _emit_transposes(*pending_vt)
                    pending_vt = None

            # ============== phase 2: attention, all heads ==================
            with tc.tile_pool(name="keephi", bufs=1) as keephip:
                keep_hi = keephip.tile([128, NKT // 2, S], BF16)
                for kt in range(NKT // 2):
                    nc.sync.dma_start(
                        out=keep_hi[:, kt, :], in_=keepT_ap[:, NKT // 2 + kt, :]
                    )

                def keep_slice(kt, q0, w):
                    t = keep_lo if kt < NKT // 2 else keep_hi
                    return t[:, kt % (NKT // 2), q0 : q0 + w]

                # deferred normalization chain (one query-group pair deep):
                # pending = (avs0, avs1, rcb, h, q0); rcb is the [2,512]
                # bf16 reciprocal-of-sums (row r = query half r)
                pending = None

                def _finish_pe(item):
                    # broadcast rcb rows onto 128 partitions via selector mms
                    avs0, avs1, rcb, h, q0 = item
                    pb0 = ps_sum.tile([128, 512], F32, tag="ps_sum", name="pb0")
                    nc.tensor.matmul(pb0, lhsT=bcL_a, rhs=rcb, start=True, stop=True)
                    pb1 = ps_sum.tile([128, 512], F32, tag="ps_sum", name="pb1")
                    nc.tensor.matmul(pb1, lhsT=bcL_b, rhs=rcb, start=True, stop=True)
                    return pb0, pb1

                def _finish_dve(item, pb0, pb1):
                    # multiply straight out of the broadcast PSUM tiles; this
                    # runs at 32-bit DVE rate but saves the two rb copies
                    avs0, avs1, rcb, h, q0 = item
                    nc.vector.tensor_mul(headsT_s[:, h, q0 : q0 + 512], avs0, pb0)
                    nc.vector.tensor_mul(
                        headsT_s[:, h, q0 + 512 : q0 + 1024], avs1, pb1
                    )

                # flat stream over (head, pair, kt) units; logits are emitted
                # one unit ahead so the PE is never the reason ACT idles at a
                # pair boundary
                PE_SET = {5, 8, 11, 15}  # key tiles summed on the PE

                def unit(u):
                    return u // 32, (u // 16) % 2, u % 16

                lg_of = {}

                def emit_lg(u):
                    h, pair, kt = unit(u)
                    q0 = pair * 1024
                    lg = ps_a.tile([128, 1024], F32, tag="ps_a", name="lg")
                    for half in range(2):
                        nc.tensor.matmul(
                            lg[:, half * 512 : (half + 1) * 512],
                            lhsT=KT_a[h][:, kt * 128 : (kt + 1) * 128],
                            rhs=QT_a[h][:, q0 + half * 512 : q0 + (half + 1) * 512],
                            start=True,
                            stop=True,
                        )
                    lg_of[u] = lg

                NU = HPC * 2 * NKT
                av0 = av1 = sm2 = accD = exD_first = None
                haveD = False
                emit_lg(0)
                for u in range(NU):
                    h, pair, kt = unit(u)
                    q0 = pair * 1024
                    if kt == 0:
                        av0 = ps_av.tile([128, 512], F32, tag="ps_av", name="av0")
                        av1 = ps_av.tile([128, 512], F32, tag="ps_av", name="av1")
                        accD = accp.tile(
                            [128, 1024], BF16, tag="accD", name="accD", bufs=2
                        )
                        exD_first = None
                        haveD = False
                    if u + 1 < NU:
                        emit_lg(u + 1)
                    if kt == 1 and pending is not None:
                        pbs = _finish_pe(pending)
                    lg = lg_of.pop(u)
                    ex = expp.tile([128, 1024], BF16, tag="ex", name="ex")
                    nc.scalar.activation(ex, lg, EXP, scale=SCALE)
                    nc.vector.tensor_mul(ex, ex, keep_slice(kt, q0, 1024))
                    if kt == 1 and pending is not None:
                        _finish_dve(pending, *pbs)
                        pending = None
                    # denominator: DVE accumulates the non-PE_SET key tiles
                    if kt not in PE_SET:
                        if exD_first is None:
                            exD_first = ex
                        elif not haveD:
                            nc.vector.tensor_add(accD, exD_first, ex)
                            haveD = True
                        else:
                            nc.vector.tensor_add(accD, accD, ex)
                    first, last = kt == 0, kt == NKT - 1
                    nc.tensor.matmul(
                        av0, lhsT=V_a[h][:, kt, :], rhs=ex[:, 0:512],
                        start=first, stop=last,
                    )
                    nc.tensor.matmul(
                        av1, lhsT=V_a[h][:, kt, :], rhs=ex[:, 512:1024],
                        start=first, stop=last,
                    )
                    if kt in PE_SET:
                        if kt == min(PE_SET):
                            # allocated late: the ps_sum ring slot must first
                            # rotate through the previous pair's pb tiles
                            sm2 = ps_sum.tile([2, 512], F32, tag="ps_sum", name="sm2")
                        nc.tensor.matmul(
                            sm2, lhsT=sumsL_a, rhs=ex[:, 0:512],
                            start=(kt == min(PE_SET)), stop=False,
                        )
                        nc.tensor.matmul(
                            sm2, lhsT=sumsL_b, rhs=ex[:, 512:1024],
                            start=False, stop=False,
                        )
                    if kt == NKT - 1:
                        # fold the DVE accumulator into the PSUM sums
                        nc.tensor.matmul(
                            sm2, lhsT=sumsL_a, rhs=accD[:, 0:512],
                            start=False, stop=False,
                        )
                        nc.tensor.matmul(
                            sm2, lhsT=sumsL_b, rhs=accD[:, 512:1024],
                            start=False, stop=True,
                        )
                        # reciprocal via ln + exp(-x), one [2,512] ACT op each
                        lnsm = smallp.tile(
                            [2, 512], F32, tag="lnsm", name="lnsm", bufs=1
                        )
                        nc.scalar.activation(lnsm, sm2, LN)
                        rcb = smallp.tile([2, 512], BF16, tag="rcb", name="rcb")
                        nc.scalar.activation(rcb, lnsm, EXP, scale=-1.0)
                        # evacuate accumulators promptly (frees PSUM banks)
                        avs0 = avsp.tile([128, 512], BF16, tag="avs", name="avs0")
                        avs1 = avsp.tile([128, 512], BF16, tag="avs", name="avs1")
                        nc.scalar.copy(avs0, av0)
                        nc.vector.tensor_copy(avs1, av1)
                        pending = (avs0, avs1, rcb, h, q0)
                if pending is not None:
                    pbs = _finish_pe(pending)
                    _finish_dve(pending, *pbs)
                    pending = None

                # ============== phase 3: output projection =================
                for qt in range(NQT):
                    po = ps_a.tile([128, 1024], F32, tag="ps_a")
                    poA, poB = po[:, 0:512], po[:, 512:1024]
                    for h in range(HPC):
                        lh = headsT_s[:, h, qt * 128 : (qt + 1) * 128]
                        for half, dst in ((0, poA), (1, poB)):
                            nc.tensor.matmul(
                                dst,
                                lhsT=lh,
                                rhs=wo_s[:, h, half * 512 : (half + 1) * 512],
                                start=(h == 0),
                                stop=(h == HPC - 1),
                            )
                    ob = outsp.tile([128, E], F16, tag="ob")
                    # split the evacuation across ACT and DVE so the PSUM
                    # banks release twice as fast
                    nc.scalar.copy(ob[:, 0:512], poA)
                    nc.vector.tensor_copy(ob[:, 512:1024], poB)
                    # two half DMAs: the first half ships while the second
                    # half is still evacuating
                    nc.sync.dma_start(
                        out=out_d[qt * 128 : (qt + 1) * 128, 0:512],
                        in_=ob[:, 0:512],
                    )
                    nc.sync.dma_start(
                        out=out_d[qt * 128 : (qt + 1) * 128, 512:1024],
                        in_=ob[:, 512:1024],
                    )

    _split_waits(nc)
    _nc_cache = nc
    return nc


def kernel(q, mask, w_query, w_key, w_value, w_out):
    nc = _build_nc()
    bf16 = ml_dtypes.bfloat16

    qT = np.ascontiguousarray(np.transpose(q.astype(bf16), (0, 2, 1)))
    keepT = np.ascontiguousarray(np.transpose((~mask).astype(bf16), (0, 2, 1)))
    wq = np.ascontiguousarray(w_query.astype(bf16))
    wk = np.ascontiguousarray(w_key.astype(bf16))
    wv = np.ascontiguousarray(w_value.astype(bf16))
    wo = np.ascontiguousarray(w_out.astype(bf16))

    in_maps = []
    for c in range(NCORES):
        b, g = c // 2, c % 2
        hs = slice(g * HPC, (g + 1) * HPC)
        in_maps.append(
            {
                "qT": qT[b],
                "keepT": keepT[b],
                "wq": wq[hs],
                "wk": wk[hs],
                "wv": wv[hs],
                "wo": wo[hs],
            }
        )

    global _last_in_maps
    _last_in_maps = in_maps
    res = run_bass_kernel_spmd(nc, in_maps, list(range(NCORES)))
    outs = [r["out"].astype(np.float32) for r in res.results]
    return np.stack([outs[2 * b] + outs[2 * b + 1] for b in range(B)])
